# revision 5
# baseline (speedup 1.0000x reference)
"""Dense u8-quantized piecewise-linear basis kernel for TRN2.

out[n, k] = relu(1 - |clip(s_n, 0, 127) - k|) * 255 as u8 on device, with
s = (x + 1) * 63.5; the host dequantizes with * (1/255). Max quantization
error 0.5/255 => rel err ~1.5e-3, well under the 2e-2 gate, and u8 cuts the
HBM write traffic 4x vs f32 (16 MiB/core).

Work split across engines (tuned on HW via rep-amplified slope benches):

  - DVE (~1 elem/cycle via the single-uop ANT_HATCLIP custom op whose
    8-stage ALU chain folds the domain clip: MIN, MAX, ABSOLUTE_DIFF,
    SUBTRACT, MULTIPLY, RELU): DVE_COLS element-columns in GROUP-column
    tiles [128, GROUP, 128] u8, one ~1 MiB DMA per tile alternating the
    SP / SWDGE(gpsimd) queues. Prep is a single tensor_scalar pass
    (s = (x+1)*63.5); the clip lives in the chain.
  - ACT: the tail ACT_C columns in a flipped layout: per column one Abs
    pass (in = knot row, bias = -clip(s_c)), and one batched Relu pass per
    ACT_BATCH columns writing contiguous [128, ACT_BATCH*128] u8. This is
    ~2x the baseline ACT path (contiguous writes + amortized instruction
    overhead); ACT runs concurrently with DVE.

The prologue is pipelined: knot/const rows load first, the ACT share of x
loads+preps before the DVE share so both engines start within ~2us. The
final DVE tile's DMA is split across both queues to shorten the tail.

Sharding: flat input axis split evenly across 8 cores (data parallel),
131072 elements/core in SBUF as [128 partitions x 1024 cols].
"""

import numpy as np

import concourse.bacc as bacc
import concourse.bass as bass
import concourse.mybir as mybir
from concourse import dve_ops
from concourse.bass_utils import run_bass_kernel_spmd
from concourse.dve_spec import (
    Bin,
    C0,
    C1,
    One,
    Spec,
    Src0,
    Src1,
    Zero,
    _has_src1,
    lower,
    maxx,
    minn,
    relu,
)
from concourse.dve_uop import AluOp as UAluOp
from concourse.dve_uop import DveOpSpec
from concourse.tile import TileContext

N = 1048576
K = 128
NCORES = 8
N_CORE = N // NCORES  # 131072
P = 128
C = N_CORE // P  # 1024 element-columns per partition
GROUP = 64  # element-columns per DVE tile / DMA chunk
DVE_COLS = 800  # columns on DVE; rest on ACT (tuned on HW, see docstring)
ACT_C = C - DVE_COLS
ACT_BATCH = 8  # columns per batched ACT Relu pass
NBUF = 6
RSTEP = 63.5
QSCALE = 255.0

F32 = mybir.dt.float32
U8 = mybir.dt.uint8
Alu = mybir.AluOpType
Act = mybir.ActivationFunctionType


def _absdiff(a, b):
    return Bin(UAluOp.ABSOLUTE_DIFF, a, b)


# in0 = knot grid k, in1 = s_raw = (x+1)*63.5 unclipped, s0 = 255, s1 = 127
# out = relu((1 - |clip(s_raw, 0, 127) - k|) * 255)
_HATCLIP_SPEC = Spec(
    body=relu((One - _absdiff(maxx(minn(Src1, C1), Zero), Src0)) * C0),
    reference=lambda in0, in1, s0, s1, imm2: np.maximum(
        (1.0 - np.abs(np.clip(in1, 0.0, s1) - in0)) * s0, 0.0
    ).astype(np.float32),
)


def _register(name: str, spec: Spec) -> dve_ops.DveOp:
    if name in dve_ops._SUB_OPCODE_FOR_NAME:
        return next(op for op in dve_ops.OPS if op.name == name)
    row = max(dve_ops._SUB_OPCODE_FOR_NAME.values()) + 1
    assert row < 0x20, row
    dve_ops._SUB_OPCODE_FOR_NAME[name] = row
    shas = {
        ver: DveOpSpec(
            name=name,
            opcode=row,
            uops=lower(spec, ver=ver),
            rd1_en=_has_src1(spec),
        ).sha(ver)
        for ver in ("v3", "v4")
    }
    op = dve_ops.DveOp(name, spec, subdim=False, uops_sha=shas)
    dve_ops.OPS.append(op)
    dve_ops.CUSTOM_DVE_SPECS[name] = spec
    return op


HATCLIP = _register("ANT_HATCLIP", _HATCLIP_SPEC)


def _build() -> bass.Bass:
    nc = bacc.Bacc("TRN2", target_bir_lowering=False, debug=False)
    xk = nc.dram_tensor("xk", [P, C + 2 * K + 2], F32, kind="ExternalInput")
    out = nc.dram_tensor("out", [N_CORE, K], U8, kind="ExternalOutput")

    out2 = out.rearrange("(p c) k -> p (c k)", p=P)  # [128, 131072] u8

    with TileContext(nc) as tc:
        with tc.tile_pool(name="persist", bufs=1) as ppool:
            xs = ppool.tile([P, C + 2 * K + 2], F32, name="xs")
            s = ppool.tile([P, C], F32, name="s")
            negs = ppool.tile([P, ACT_C], F32, name="negs")
            tmpa = ppool.tile([P, ACT_BATCH * K], F32, name="tmpa")
            ta = tmpa[:].rearrange("p (b k) -> p b k", b=ACT_BATCH)
            act_out = ppool.tile([P, ACT_C * K], U8, name="act_out")
            av = act_out[:].rearrange("p (c k) -> p c k", c=ACT_C)
            bufs = [
                ppool.tile([P, GROUP * K], U8, name=f"b{i}") for i in range(NBUF)
            ]

            # knot/const rows first (needed by both engines)
            nc.sync.dma_start(out=xs[:, C:], in_=xk[:, C:])
            kn = xs[:, C : C + K]
            b255 = xs[:, C + 2 * K : C + 2 * K + 1]
            b63 = xs[:, C + 2 * K + 1 : C + 2 * K + 2]
            in0 = kn.unsqueeze(1).broadcast_to([P, GROUP, K])

            # chunked x load + one-pass prep: s_raw = (x + 1) * 63.5
            def prep(lo, hi):
                nc.gpsimd.dma_start(out=xs[:, lo:hi], in_=xk[:, lo:hi])
                nc.vector.tensor_scalar(
                    s[:, lo:hi], xs[:, lo:hi], 1.0, RSTEP, Alu.add, Alu.mult
                )

            def prep_act(lo, hi):
                # s = 63.5*x + 63.5 on ACT (Identity, AP bias) — keeps the
                # prep off the critical DVE stream; ACT has slack.
                nc.gpsimd.dma_start(out=xs[:, lo:hi], in_=xk[:, lo:hi])
                nc.scalar.activation(
                    s[:, lo:hi], xs[:, lo:hi], Act.Identity, bias=b63,
                    scale=RSTEP,
                )

            prep(DVE_COLS, C)  # ACT share first so ACT starts early
            # negs = -clip(s_raw, 0, 127) for the ACT bias
            nc.vector.tensor_scalar(
                negs, s[:, DVE_COLS:C], 127.0, 0.0, Alu.min, Alu.max
            )
            nc.vector.tensor_scalar(negs, negs, -1.0, 0.0, Alu.mult, Alu.add)
            prep_act(0, GROUP)
            prep_act(GROUP, DVE_COLS)

            qs = [nc.sync, nc.gpsimd]
            ndve = (DVE_COLS + GROUP - 1) // GROUP

            # --- DVE share (last group may be partial) ------------------
            for g in range(ndve):
                c0 = g * GROUP
                cw = min(GROUP, DVE_COLS - c0)
                B = bufs[g % NBUF]
                in1 = s[:, c0 : c0 + cw].unsqueeze(2).broadcast_to([P, cw, K])
                o3 = B[:, : cw * K].rearrange("p (g k) -> p g k", g=cw)
                i0 = (
                    in0
                    if cw == GROUP
                    else kn.unsqueeze(1).broadcast_to([P, cw, K])
                )
                nc.vector._custom_dve(
                    HATCLIP, out=o3, in0=i0, in1=in1, s0=QSCALE, s1=127.0
                )
                if g == ndve - 1:
                    # split the final DMA across both queues to cut the tail
                    half = cw * K // 2
                    qs[0].dma_start(
                        out=out2[:, c0 * K : c0 * K + half], in_=B[:, :half]
                    )
                    qs[1].dma_start(
                        out=out2[:, c0 * K + half : (c0 + cw) * K],
                        in_=B[:, half : cw * K],
                    )
                else:
                    qs[g % 2].dma_start(
                        out=out2[:, c0 * K : (c0 + cw) * K], in_=B[:, : cw * K]
                    )

            # --- ACT share (flipped layout, batched Relu pass) ----------
            for ci in range(ACT_C):
                b = ci % ACT_BATCH
                nc.scalar.activation(
                    ta[:, b, :], kn, Act.Abs, bias=negs[:, ci : ci + 1]
                )
                if b == ACT_BATCH - 1:
                    nc.scalar.activation(
                        av[:, ci - ACT_BATCH + 1 : ci + 1, :],
                        ta,
                        Act.Relu,
                        bias=b255,
                        scale=-QSCALE,
                    )
                if (ci + 1) % GROUP == 0 or ci == ACT_C - 1:
                    gg = ci // GROUP
                    lo = (DVE_COLS + gg * GROUP) * K
                    hi = (DVE_COLS + ci + 1) * K
                    qs[gg % 2].dma_start(
                        out=out2[:, lo:hi],
                        in_=act_out[:, gg * GROUP * K : (ci + 1) * K],
                    )
    nc.finalize()
    return nc


def _in_maps(x: np.ndarray) -> list[dict]:
    knots = np.arange(K, dtype=np.float32)[None, :]
    extra = np.concatenate(
        [
            knots,
            -knots,
            np.full((1, 1), QSCALE, np.float32),
            np.full((1, 1), RSTEP, np.float32),
        ],
        axis=1,
    )
    extra = np.broadcast_to(extra, (P, 2 * K + 2))
    shards = x.reshape(NCORES, P, C)
    return [
        {"xk": np.ascontiguousarray(np.concatenate([shards[i], extra], axis=1))}
        for i in range(NCORES)
    ]


def _dequant(u8: np.ndarray) -> np.ndarray:
    return u8.astype(np.float32) * np.float32(1.0 / QSCALE)


def kernel(inputs: np.ndarray, num_knots) -> np.ndarray:
    assert int(num_knots) == K, f"kernel hardcoded for num_knots={K}"
    x = np.ascontiguousarray(np.asarray(inputs, dtype=np.float32))
    assert x.shape == (N,), x.shape

    nc = _build()
    res = run_bass_kernel_spmd(nc, _in_maps(x), core_ids=list(range(NCORES)))
    return _dequant(np.concatenate([r["out"] for r in res.results], axis=0))
